# revision 1
# baseline (speedup 1.0000x reference)
"""Luong attention (dot-product attention with per-position scale) on 8 TRN2 cores.

Full-input contract: kernel(query[32,1024], values[32,4096,1024], scale[4096,1])
-> context[32,1024].  Batch is sharded 4-per-core across 8 NeuronCores
(data-parallel, no collectives).

Per-core plan (B=4 batches, S=4096, H=1024):
  - V[b] streamed HBM->SBUF exactly once, partition-major s-layout
    (s = p*32 + j) so every partition reads contiguous 16 KiB runs;
    2 MiB per dma_start.
  - scores[s] = sum_h V[s,h]*q[h] (exact fp32) via the fused DVE
    scalar_tensor_tensor (mult + free-axis sum accumulator) against a
    partition-replicated q (built by a ones-outer-product on the PE).
  - scores *= scale[s]; softmax statistics per S-half: free-axis max (DVE)
    -> partition all-reduce max (GpSimd, result replicated) -> Exp with
    fused row-sum on ScalarE (E emitted directly in bf16) -> denominator
    via ones-matmul on PE.
  - context = sum_s E[s]*V[s,:] on PE in bf16 (E column stationary, V
    moving, PSUM-accumulated); V is cast fp32->bf16 on the otherwise-idle
    ScalarE (fp32 PE matmuls lower to two HW passes -- 4x slower).
  - S is split asymmetrically (3072 + 1024 positions); each part runs an
    independent local softmax and the parts merge with the flash-attention
    rescale, so the big part's weighted sum overlaps the small part's
    loads and only ~16 matmuls + one merge remain after the last DMA.
Per-core HBM traffic ~64 MiB -> ~190 us roofline; measured ~220 us.
Scores/softmax are fp32-exact; only the final weighted average uses bf16
(~3e-3 max-rel output error).
"""

import sys

sys.path.insert(0, "/opt/trn_rl_repo")

from contextlib import ExitStack

import numpy as np

import concourse.bacc as bacc
import concourse.tile as tile
from concourse import bass_isa, mybir
from concourse.bass_utils import run_bass_kernel_spmd

F32 = mybir.dt.float32
BF16 = mybir.dt.bfloat16

N_CORES = 8
B_FULL = 32
S = 4096
H = 1024
B_PER_CORE = B_FULL // N_CORES  # 4

P = 128               # partitions
N_CHUNK = S // P      # 32 s-slots per partition; s = p*32 + j (partition-major)
CHUNKS_PER_DMA = 4    # 2 MiB per dma_start, 16 KiB contiguous per partition
N_DMA_GROUPS = N_CHUNK // CHUNKS_PER_DMA  # 8
VBUFS = 4             # fp32 staging slots (16 KiB/partition; freed after scores+cast)
BBUFS = 11            # bf16 V slots (8 KiB/partition; live until weighted sum)


def build_kernel(nb=B_PER_CORE, n_chunk=N_CHUNK, vbufs=VBUFS, bbufs=BBUFS):
    s = n_chunk * P
    nc = bacc.Bacc("TRN2", target_bir_lowering=False, debug=False)

    q_d = nc.dram_tensor("query", (nb, H), F32, kind="ExternalInput")
    v_d = nc.dram_tensor("values", (nb, s, H), F32, kind="ExternalInput")
    scale_d = nc.dram_tensor("scale", (s, 1), F32, kind="ExternalInput")
    out_d = nc.dram_tensor("out", (nb, H), F32, kind="ExternalOutput")

    n_groups = n_chunk // CHUNKS_PER_DMA

    with tile.TileContext(nc) as tc, ExitStack() as ctx:
        consts = ctx.enter_context(tc.tile_pool(name="consts", bufs=1))
        vpool = ctx.enter_context(tc.tile_pool(name="vpool", bufs=vbufs))
        bpool = ctx.enter_context(tc.tile_pool(name="bpool", bufs=bbufs))
        qpool = ctx.enter_context(tc.tile_pool(name="qpool", bufs=2))
        spool = ctx.enter_context(tc.tile_pool(name="spool", bufs=2))
        scratch = ctx.enter_context(tc.tile_pool(name="scratch", bufs=2))
        opool = ctx.enter_context(tc.tile_pool(name="opool", bufs=2))
        psum = ctx.enter_context(tc.tile_pool(name="psum", bufs=2, space="PSUM"))
        zpsum = ctx.enter_context(tc.tile_pool(name="zpsum", bufs=2, space="PSUM"))
        qps = ctx.enter_context(tc.tile_pool(name="qps", bufs=1, space="PSUM"))

        # ---- one-time constants ----
        ones_col = consts.tile([P, 1], F32)
        nc.vector.memset(ones_col, 1.0)
        ones_row = consts.tile([1, P], F32)
        nc.vector.memset(ones_row, 1.0)

        # scale[s] -> scale_sb[p, j] with s = p*n_chunk + j (partition-major,
        # matching the V layout below) -- a direct strided DMA, no transpose.
        # (scalar-engine HWDGE queue, so it doesn't delay the V loads)
        scale_sb = consts.tile([P, n_chunk], F32)
        nc.scalar.dma_start(
            out=scale_sb[:],
            in_=scale_d.rearrange("(p j) o -> p (j o)", p=P),
        )

        for b in range(nb):
            # ---- replicate q[b] across 128 partitions: 4 KiB DMA, then a
            # ones-outer-product on PE (PSUM) evacuated by ScalarE ----
            q_sb = qpool.tile([1, H], F32, tag="q_sb")
            nc.scalar.dma_start(out=q_sb[:], in_=q_d[b : b + 1, :])
            q_ps = qps.tile([P, H], F32, tag="q_ps")
            q_rep = qpool.tile([P, H], F32)
            for h0 in range(0, H, 512):
                nc.tensor.matmul(q_ps[:, h0 : h0 + 512], lhsT=ones_row[:],
                                 rhs=q_sb[:, h0 : h0 + 512],
                                 start=True, stop=True)
                nc.scalar.copy(out=q_rep[:, h0 : h0 + 512],
                               in_=q_ps[:, h0 : h0 + 512])

            # ---- stream V[b] in two S-halves, each with its own local
            # softmax (max m_h, denominator Z_h, unnormalized context
            # ctxu_h); the halves merge with the standard flash-attention
            # rescale.  Half A's weighted sum runs while half B still loads,
            # halving the post-DMA tail. ----
            v_view = v_d[b].rearrange("(p j) h -> p j h", p=P)
            # Asymmetric S-split: the big part's weighted sum overlaps the
            # small part's loads; only the small part's matmuls + one merge
            # remain after the last DMA.
            parts = [(0, 3 * n_groups // 4), (3 * n_groups // 4, n_groups)]
            ctxu = []
            msb = []
            zsb = []
            for hf, (g0, g1) in enumerate(parts):
                nh = (g1 - g0) * CHUNKS_PER_DMA
                j0 = g0 * CHUNKS_PER_DMA
                vbtiles = []
                scores = spool.tile([P, nh], F32, tag="scores")
                for gl in range(g1 - g0):
                    g = g0 + gl
                    vt = vpool.tile([P, CHUNKS_PER_DMA, H], F32, tag="vt")
                    nc.sync.dma_start(
                        out=vt[:],
                        in_=v_view[:, g * CHUNKS_PER_DMA : (g + 1) * CHUNKS_PER_DMA, :],
                    )
                    # bf16 copy for the weighted-sum matmuls (ScalarE is
                    # idle); fp32 staging frees once scores + cast are done.
                    vb = bpool.tile([P, CHUNKS_PER_DMA, H], BF16, tag="vb")
                    nc.scalar.copy(out=vb[:], in_=vt[:])
                    vbtiles.append(vb)
                    for cl in range(CHUNKS_PER_DMA):
                        c = gl * CHUNKS_PER_DMA + cl
                        prod = scratch.tile([P, H], F32, tag="prod")
                        # scale[s] folded in via the per-partition scalar
                        # slot: accum = sum_h (V*scale_s)*q = scale_s*score
                        nc.vector.scalar_tensor_tensor(
                            out=prod[:],
                            in0=vt[:, cl, :],
                            scalar=scale_sb[:, j0 + c : j0 + c + 1],
                            in1=q_rep[:],
                            op0=mybir.AluOpType.mult,
                            op1=mybir.AluOpType.mult,
                            accum_out=scores[:, c : c + 1],
                        )

                # local softmax pieces for this half (scores already scaled)
                scores2 = scores
                m1 = spool.tile([P, 1], F32, tag="m1")
                nc.vector.tensor_reduce(
                    out=m1[:], in_=scores2[:],
                    axis=mybir.AxisListType.X, op=mybir.AluOpType.max,
                )
                m_all = spool.tile([P, 1], F32, tag="m_all")
                nc.gpsimd.partition_all_reduce(
                    out_ap=m_all[:], in_ap=m1[:], channels=P,
                    reduce_op=bass_isa.ReduceOp.max,
                )
                negm = spool.tile([P, 1], F32, tag="negm")
                nc.scalar.mul(negm[:], m_all[:], -1.0)

                e_t = spool.tile([P, nh], BF16, tag="e_t")
                s1 = spool.tile([P, 1], F32, tag="s1")
                nc.scalar.activation(
                    out=e_t[:], in_=scores2[:],
                    func=mybir.ActivationFunctionType.Exp,
                    bias=negm[:], scale=1.0,
                    accum_out=s1[:],
                )

                z_ps = zpsum.tile([1, 1], F32, tag="z")
                nc.tensor.matmul(z_ps[:], lhsT=s1[:], rhs=ones_col[:],
                                 start=True, stop=True)
                z_sb = spool.tile([1, 1], F32, tag="z_sb")
                nc.vector.tensor_copy(z_sb[:], z_ps[:])
                zsb.append(z_sb)
                msb.append(m_all[0:1, :])

                ctx_ps = psum.tile([1, H], F32, tag="ctx")
                for c in range(nh):
                    vb = vbtiles[c // CHUNKS_PER_DMA]
                    cl = c % CHUNKS_PER_DMA
                    for h0 in range(0, H, 512):
                        nc.tensor.matmul(
                            ctx_ps[:, h0 : h0 + 512],
                            lhsT=e_t[:, c : c + 1],
                            rhs=vb[:, cl, h0 : h0 + 512],
                            start=(c == 0),
                            stop=(c == nh - 1),
                        )
                ctxu.append(ctx_ps)

            # ---- merge halves: m = max(mA,mB); wH = exp(mH - m);
            # Z = wA*ZA + wB*ZB; ctx = (wA*ctxuA + wB*ctxuB)/Z ----
            mm = spool.tile([1, 1], F32, tag="mm")
            nc.vector.tensor_tensor(out=mm[:], in0=msb[0], in1=msb[1],
                                    op=mybir.AluOpType.max)
            negmm = spool.tile([1, 1], F32, tag="negmm")
            nc.scalar.mul(negmm[:], mm[:], -1.0)
            w_h = []
            for hf in range(2):
                w = spool.tile([1, 1], F32, tag=f"w{hf}")
                nc.scalar.activation(
                    out=w[:], in_=msb[hf],
                    func=mybir.ActivationFunctionType.Exp,
                    bias=negmm[:], scale=1.0,
                )
                w_h.append(w)
            za = spool.tile([1, 1], F32, tag="za")
            nc.vector.tensor_mul(za[:], zsb[0][:], w_h[0][:])
            zz = spool.tile([1, 1], F32, tag="zz")
            nc.vector.scalar_tensor_tensor(
                out=zz[:], in0=zsb[1][:], scalar=w_h[1][:], in1=za[:],
                op0=mybir.AluOpType.mult, op1=mybir.AluOpType.add,
            )
            r_sb = spool.tile([1, 1], F32, tag="r")
            nc.vector.reciprocal(out=r_sb[:], in_=zz[:])
            sA = spool.tile([1, 1], F32, tag="sA")
            nc.vector.tensor_mul(sA[:], w_h[0][:], r_sb[:])
            sB = spool.tile([1, 1], F32, tag="sB")
            nc.vector.tensor_mul(sB[:], w_h[1][:], r_sb[:])

            t1 = opool.tile([1, H], F32, tag="t1")
            nc.scalar.mul(t1[:], ctxu[0][:], sA[:])
            ctx_sb = opool.tile([1, H], F32, tag="ctx_sb")
            nc.vector.scalar_tensor_tensor(
                out=ctx_sb[:], in0=ctxu[1][:], scalar=sB[:], in1=t1[:],
                op0=mybir.AluOpType.mult, op1=mybir.AluOpType.add,
            )
            nc.sync.dma_start(out=out_d[b : b + 1, :], in_=ctx_sb[:])

    nc.compile()
    return nc


_NC_CACHE = {}


def _get_nc():
    if "nc" not in _NC_CACHE:
        _NC_CACHE["nc"] = build_kernel()
    return _NC_CACHE["nc"]


def run(query, values, scale, trace=False, **kw):
    nc = _get_nc()
    query = np.ascontiguousarray(query, dtype=np.float32)
    values = np.ascontiguousarray(values, dtype=np.float32)
    scale = np.ascontiguousarray(scale, dtype=np.float32)
    in_maps = []
    for core in range(N_CORES):
        lo = core * B_PER_CORE
        hi = lo + B_PER_CORE
        in_maps.append(
            {"query": query[lo:hi], "values": values[lo:hi], "scale": scale}
        )
    res = run_bass_kernel_spmd(nc, in_maps, core_ids=list(range(N_CORES)),
                               trace=trace, **kw)
    out = np.concatenate([r["out"] for r in res.results], axis=0)
    return out, res


def kernel(query, values, scale):
    out, _ = run(query, values, scale)
    return out.astype(np.float32)

